# revision 1
# baseline (speedup 1.0000x reference)
"""DecoupledContrastiveLoss on 8 Trainium2 NeuronCores.

Strategy (data parallel over batch rows, per sharding hint):
  - Host: stable-sort rows by match_id (makes the positive mask a narrow
    band around the diagonal), L2-normalize rows, transpose both feature
    matrices to [D, B] so the contraction dim lands on SBUF partitions,
    and ship each core a column-rotated copy (rotation by core*1024 puts
    the core's own diagonal block at local columns [0, 1024), so one SPMD
    program serves all cores).
  - Device (per core, fp32r matmuls): 4 row-sharded [1024, 8192] similarity
    passes (v2t, t2v, v@v.T, t@t.T). Each sim chunk goes PSUM -> ACT
    exp(x/T) with fused row-sum accumulation. DVE computes top-8
    max+indices per half-row (v2t/t2v) and the masked positive sums over
    the 256-wide diagonal band (is_equal vs ids + multiply-reduce).
    Instance passes extract exp(diag) via an identity multiply-reduce.
  - Host: combines per-core/per-half partials, computes the log-space
    losses, refines argmax among the 16 device candidates with exact
    dots, and assembles the 9 reference outputs.
"""
import sys

if "/opt/trn_rl_repo" not in sys.path:
    sys.path.insert(0, "/opt/trn_rl_repo")

import numpy as np

import concourse.bacc as bacc
import concourse.tile as tile
import concourse.mybir as mybir
from concourse.bass_utils import run_bass_kernel_spmd

DT = mybir.dt

N_CORES = 8
B = 8192
D = 512
BL = B // N_CORES          # 1024 rows per core
NT = BL // 128             # 8 i-tiles per core
HALF = B // 2              # 4096 columns per phase
TEMP = 0.07
T_INV = 1.0 / TEMP
WIN = 256                  # positive-band window width (max group size 8 << 64)

_program = None
_last_in_maps = None


def _build_program(repeat=1, mov_bufs=8, e_bufs=2, es_bufs=2, ps_bufs=4):
    nc = bacc.Bacc("TRN2", target_bir_lowering=False, debug=False,
                   num_devices=N_CORES)

    vmov = nc.dram_tensor("vmov", [D, B], DT.float32r, kind="ExternalInput").ap()
    tmov = nc.dram_tensor("tmov", [D, B], DT.float32r, kind="ExternalInput").ap()
    ids_win = nc.dram_tensor("ids_win", [NT, WIN], DT.float32, kind="ExternalInput").ap()
    ids_loc = nc.dram_tensor("ids_loc", [128, NT], DT.float32, kind="ExternalInput").ap()
    ident = nc.dram_tensor("ident", [128, 128], DT.float32, kind="ExternalInput").ap()

    def out_t(name, w, dtype=DT.float32):
        return nc.dram_tensor(name, [BL, w], dtype, kind="ExternalOutput").ap()

    outs = {}
    for nm in ("v2t", "t2v"):
        outs[nm + "_tot"] = out_t(nm + "_tot", 2)
        outs[nm + "_pos"] = out_t(nm + "_pos", 2)
        outs[nm + "_max"] = out_t(nm + "_max", 16)
        outs[nm + "_idx"] = out_t(nm + "_idx", 16, DT.uint32)
    for nm in ("vv", "tt"):
        outs[nm + "_tot"] = out_t(nm + "_tot", 2)
        outs[nm + "_diag"] = out_t(nm + "_diag", 1)

    with tile.TileContext(nc) as tc:
        with tc.tile_pool(name="consts", bufs=1) as cpool, \
             tc.tile_pool(name="mov", bufs=mov_bufs) as mpool, \
             tc.tile_pool(name="eblk", bufs=e_bufs) as epool, \
             tc.tile_pool(name="esc", bufs=es_bufs) as escpool, \
             tc.tile_pool(name="small", bufs=3) as spool, \
             tc.tile_pool(name="gmp", bufs=2) as gmpool, \
             tc.tile_pool(name="psum", bufs=ps_bufs, space="PSUM") as pspool:

            def load_mov(mat, half):
                mov_dram = tmov if mat == "t" else vmov
                mk = [mpool.tile([128, HALF], DT.float32r, name="movk")
                      for _ in range(4)]
                # q-outer: the first 512-col chunk needs all four k slices,
                # so land the q=0 pieces of every k first
                for q in range(4):
                    for k in range(4):
                        nc.sync.dma_start(
                            mk[k][:, q * 1024:(q + 1) * 1024],
                            mov_dram[k * 128:(k + 1) * 128,
                                     half * HALF + q * 1024:
                                     half * HALF + (q + 1) * 1024])
                return mk

            # phases: (moving matrix, half)
            phases = [("t", 0), ("t", 1), ("v", 0), ("v", 1)] * repeat

            # critical path first: cross stationary (vloc) + phase-0 moving
            vloc = cpool.tile([128, 4 * BL], DT.float32r)
            tloc = cpool.tile([128, 4 * BL], DT.float32r)
            for k in range(4):
                nc.sync.dma_start(vloc[:, k * BL:(k + 1) * BL],
                                  vmov[k * 128:(k + 1) * 128, 0:BL])
            mk0 = load_mov(*phases[0])
            for k in range(4):
                nc.sync.dma_start(tloc[:, k * BL:(k + 1) * BL],
                                  tmov[k * 128:(k + 1) * 128, 0:BL])

            win = cpool.tile([128, NT * WIN], DT.float32)
            for it in range(NT):
                nc.gpsimd.dma_start(win[:, it * WIN:(it + 1) * WIN],
                                    ids_win[it:it + 1, :].partition_broadcast(128))
            idl = cpool.tile([128, NT], DT.float32)
            nc.gpsimd.dma_start(idl[:], ids_loc[:])
            idn = cpool.tile([128, 128], DT.float32)
            nc.gpsimd.dma_start(idn[:], ident[:])
            for pi, (mat, half) in enumerate(phases):
                cross = "v2t" if mat == "t" else "t2v"
                inst = "tt" if mat == "t" else "vv"
                cstat = vloc if mat == "t" else tloc
                istat = tloc if mat == "t" else vloc

                mk = mk0 if pi == 0 else load_mov(mat, half)

                def mm_group(pp, stat, it, g):
                    # fill [128, 1024] psum group g of i-tile it
                    for cc in range(2):
                        for k in range(4):
                            nc.tensor.matmul(
                                pp[:, cc * 512:(cc + 1) * 512],
                                stat[:, k * BL + it * 128: k * BL + it * 128 + 128],
                                mk[k][:, g * 1024 + cc * 512: g * 1024 + (cc + 1) * 512],
                                start=(k == 0), stop=(k == 3))

                for it in range(NT):
                    # ---------- cross-modal i-tile (needs max/idx + pos) ----
                    e = epool.tile([128, HALF], DT.bfloat16, name="e")
                    tp = spool.tile([128, 4], DT.float32, name="tp")
                    for g in range(4):
                        pp = pspool.tile([128, 1024], DT.float32, name="pp")
                        mm_group(pp, cstat, it, g)
                        nc.scalar.activation(
                            e[:, g * 1024:(g + 1) * 1024], pp[:],
                            mybir.ActivationFunctionType.Exp,
                            bias=0.0, scale=T_INV, accum_out=tp[:, g:g + 1])
                    tot1 = spool.tile([128, 1], DT.float32, name="tot1")
                    nc.vector.tensor_reduce(tot1[:], tp[:],
                                            axis=mybir.AxisListType.X,
                                            op=mybir.AluOpType.add)
                    nc.gpsimd.dma_start(
                        outs[cross + "_tot"][it * 128:(it + 1) * 128, half:half + 1],
                        tot1[:])
                    # two-level argmax: 8-wide group maxes, then top-8 groups
                    gm = gmpool.tile([128, HALF // 8], DT.float32, name="gm")
                    nc.vector.tensor_reduce(
                        gm[:], e[:].rearrange("p (g k) -> p g k", k=8),
                        axis=mybir.AxisListType.X, op=mybir.AluOpType.max)
                    mx = spool.tile([128, 8], DT.float32, name="mx")
                    ix = spool.tile([128, 8], DT.uint32, name="ix")
                    nc.vector.max_with_indices(mx[:], ix[:], gm[:])
                    nc.gpsimd.dma_start(
                        outs[cross + "_max"][it * 128:(it + 1) * 128,
                                             half * 8:(half + 1) * 8], mx[:])
                    nc.gpsimd.dma_start(
                        outs[cross + "_idx"][it * 128:(it + 1) * 128,
                                             half * 8:(half + 1) * 8], ix[:])

                    # positive band: local cols [it*128-64, it*128+192) mod B
                    def mask_pos(e_lo, e_hi, w_lo, pos_col):
                        width = e_hi - e_lo
                        msk = spool.tile([128, WIN], DT.float32, name="msk")
                        junk = spool.tile([128, WIN], DT.float32, name="junk")
                        pos1 = spool.tile([128, 1], DT.float32, name="pos1")
                        nc.vector.tensor_scalar(
                            msk[:, 0:width],
                            win[:, it * WIN + w_lo: it * WIN + w_lo + width],
                            idl[:, it:it + 1], None,
                            op0=mybir.AluOpType.is_equal)
                        nc.vector.tensor_tensor(
                            junk[:, 0:width], e[:, e_lo:e_hi], msk[:, 0:width],
                            op=mybir.AluOpType.mult)
                        nc.vector.tensor_reduce(
                            pos1[:], junk[:, 0:width],
                            axis=mybir.AxisListType.X, op=mybir.AluOpType.add)
                        nc.gpsimd.dma_start(
                            outs[cross + "_pos"][it * 128:(it + 1) * 128,
                                                 pos_col:pos_col + 1], pos1[:])

                    if half == 0:
                        if it == 0:
                            mask_pos(0, 192, 64, 0)       # cols [0, 192)
                        else:
                            mask_pos(it * 128 - 64, it * 128 + 192, 0, 0)
                    elif it == 0:
                        mask_pos(HALF - 64, HALF, 0, 1)    # wrap: cols [B-64, B)

                    # ---------- instance i-tile (tot + diag only) ----------
                    tpi = spool.tile([128, 4], DT.float32, name="tpi")
                    for g in range(4):
                        pp = pspool.tile([128, 1024], DT.float32, name="pp")
                        mm_group(pp, istat, it, g)
                        es = escpool.tile([128, 1024], DT.float32, name="es")
                        nc.scalar.activation(
                            es[:], pp[:], mybir.ActivationFunctionType.Exp,
                            bias=0.0, scale=T_INV, accum_out=tpi[:, g:g + 1])
                        if half == 0 and g == 0:
                            junkd = spool.tile([128, 128], DT.float32, name="junkd")
                            diag1 = spool.tile([128, 1], DT.float32, name="diag1")
                            nc.vector.tensor_tensor(
                                junkd[:], es[:, it * 128:it * 128 + 128], idn[:],
                                op=mybir.AluOpType.mult)
                            nc.vector.tensor_reduce(
                                diag1[:], junkd[:],
                                axis=mybir.AxisListType.X, op=mybir.AluOpType.add)
                            nc.gpsimd.dma_start(
                                outs[inst + "_diag"][it * 128:(it + 1) * 128, 0:1],
                                diag1[:])
                    toti = spool.tile([128, 1], DT.float32, name="toti")
                    nc.vector.tensor_reduce(toti[:], tpi[:],
                                            axis=mybir.AxisListType.X,
                                            op=mybir.AluOpType.add)
                    nc.gpsimd.dma_start(
                        outs[inst + "_tot"][it * 128:(it + 1) * 128, half:half + 1],
                        toti[:])
    nc.compile()
    return nc


def _get_program():
    global _program
    if _program is None:
        _program = _build_program()
    return _program


def kernel(vision_features, text_features, match_ids):
    v = np.asarray(vision_features, dtype=np.float32)
    t = np.asarray(text_features, dtype=np.float32)
    ids = np.asarray(match_ids)

    # ---- host prep: sort by id, normalize, transpose ----
    perm = np.argsort(ids, kind="stable")
    ids_s = ids[perm].astype(np.int64)
    v_s = v[perm]
    t_s = t[perm]
    vn = (v_s / np.linalg.norm(v_s, axis=1, keepdims=True)).astype(np.float32)
    tn = (t_s / np.linalg.norm(t_s, axis=1, keepdims=True)).astype(np.float32)
    vT = np.ascontiguousarray(vn.T)   # [D, B]
    tT = np.ascontiguousarray(tn.T)

    # group ranges in sorted order
    ids_f = ids_s.astype(np.float32)
    change = np.nonzero(np.diff(ids_s))[0] + 1
    starts = np.concatenate([[0], change])
    ends = np.concatenate([change, [B]])
    cnt = ends - starts
    num_pos = int((cnt.astype(np.int64) ** 2).sum())
    assert cnt.max() <= 64, "positive band wider than window"

    in_maps = []
    for d in range(N_CORES):
        sl = slice(d * BL, (d + 1) * BL)
        vrot = np.roll(vT, -d * BL, axis=1)
        trot = np.roll(tT, -d * BL, axis=1)
        ids_win = np.empty((NT, WIN), np.float32)
        for it in range(NT):
            cols = (np.arange(it * 128 - 64, it * 128 + 192) + d * BL) % B
            ids_win[it] = ids_f[cols]
        ids_loc = ids_f[sl].reshape(NT, 128).T.copy()  # [128, NT]
        in_maps.append({
            "vmov": vrot, "tmov": trot,
            "ids_win": ids_win, "ids_loc": ids_loc,
            "ident": np.eye(128, dtype=np.float32),
        })

    global _last_in_maps
    _last_in_maps = in_maps
    nc = _get_program()
    res = run_bass_kernel_spmd(nc, in_maps, list(range(N_CORES)))

    def gather(name):
        return np.concatenate([res.results[c][name] for c in range(N_CORES)], axis=0)

    out = {k: gather(k) for k in
           ["v2t_tot", "v2t_pos", "v2t_max", "v2t_idx",
            "t2v_tot", "t2v_pos", "t2v_max", "t2v_idx",
            "vv_tot", "vv_diag", "tt_tot", "tt_diag"]}

    # ---- losses (all rows valid: every row has >=1 pos and >=1 neg) ----
    f64 = np.float64
    v2t_tot = out["v2t_tot"].astype(f64).sum(1)
    t2v_tot = out["t2v_tot"].astype(f64).sum(1)

    def pos_sum(a):
        # column 1 is only written for each core's first i-tile (the band
        # wrap); everything else is undefined memory — mask it out.
        s = a.astype(f64)[:, 0].copy()
        for c in range(N_CORES):
            lo = c * BL
            s[lo:lo + 128] += a[lo:lo + 128, 1].astype(f64)
        return s

    v2t_pos = pos_sum(out["v2t_pos"])
    t2v_pos = pos_sum(out["t2v_pos"])
    v2t_loss = (np.log(v2t_tot) - np.log(v2t_pos)).sum() / num_pos
    t2v_loss = (np.log(t2v_tot) - np.log(t2v_pos)).sum() / num_pos
    cross = 0.5 * (v2t_loss + t2v_loss)

    vv_tot = out["vv_tot"].astype(f64).sum(1)
    tt_tot = out["tt_tot"].astype(f64).sum(1)
    v_inst = (np.log(vv_tot) - np.log(out["vv_diag"].astype(f64)[:, 0])).mean()
    t_inst = (np.log(tt_tot) - np.log(out["tt_diag"].astype(f64)[:, 0])).mean()

    total = cross + 0.5 * v_inst + 0.5 * t_inst

    # ---- accuracy: refine argmax among the 16 device candidates ----
    core_of_row = np.repeat(np.arange(N_CORES), BL)

    def refine(idx, a_s, b_s):
        # idx: [B, 16] top-8 group indices per half (groups of 8 columns);
        # expand to the 128 member columns and take the exact-fp32 argmax.
        gidx = idx.astype(np.int64)
        gidx[:, 8:] += HALF // 8
        loc = (gidx[:, :, None] * 8 + np.arange(8)).reshape(B, 128)
        g = (loc + core_of_row[:, None] * BL) % B      # global sorted col
        best = np.empty(B, np.int64)
        for lo in range(0, B, 512):
            hi = lo + 512
            cand = b_s[g[lo:hi]]                       # [512, 128, D]
            sims = np.matmul(cand, a_s[lo:hi, :, None])[:, :, 0]
            best[lo:hi] = g[np.arange(lo, hi), sims.argmax(1)]
        return best

    v2t_pred_s = refine(out["v2t_idx"], vn, tn)
    t2v_pred_s = refine(out["t2v_idx"], tn, vn)

    # map sorted-space preds back to original indexing
    ids_orig = ids.astype(np.int64)
    order = np.argsort(ids_orig, kind="stable")
    first_occ_sorted_pos = np.searchsorted(ids_orig[order], ids_orig)
    targets = order[first_occ_sorted_pos]              # first orig idx with same id

    pred_v2t = np.empty(B, np.int64)
    pred_v2t[perm] = perm[v2t_pred_s]
    pred_t2v = np.empty(B, np.int64)
    pred_t2v[perm] = perm[t2v_pred_s]
    v2t_acc = (pred_v2t == targets).mean()
    t2v_acc = (pred_t2v == targets).mean()

    r = np.float32
    return (r(total), r(cross), r(v2t_loss), r(t2v_loss),
            r(v_inst), r(t_inst), r(v2t_acc), r(t2v_acc),
            r((v2t_acc + t2v_acc) / 2.0))



# revision 7
# speedup vs baseline: 1.4447x; 1.4447x over previous
"""DecoupledContrastiveLoss on 8 Trainium2 NeuronCores.

Strategy (data parallel over batch rows, per sharding hint):
  - Host: stable-sort rows by match_id (makes the positive mask a narrow
    band around the diagonal), L2-normalize rows, transpose both feature
    matrices to [D, B] so the contraction dim lands on SBUF partitions,
    and ship each core a column-rotated copy (rotation by core*1024 puts
    the core's own diagonal block at local columns [0, 1024), so one SPMD
    program serves all cores).
  - Device (per core, fp32r matmuls): 4 row-sharded [1024, 8192] similarity
    passes (v2t, t2v, v@v.T, t@t.T). Each sim chunk goes PSUM -> ACT
    exp(x/T) with fused row-sum accumulation. DVE computes top-8
    max+indices per half-row (v2t/t2v) and the masked positive sums over
    the 256-wide diagonal band (is_equal vs ids + multiply-reduce).
    Instance passes extract exp(diag) via an identity multiply-reduce.
  - Host: combines per-core/per-half partials, computes the log-space
    losses, refines argmax among the 16 device candidates with exact
    dots, and assembles the 9 reference outputs.
"""
import sys

if "/opt/trn_rl_repo" not in sys.path:
    sys.path.insert(0, "/opt/trn_rl_repo")

import numpy as np

import concourse.bacc as bacc
import concourse.tile as tile
import concourse.mybir as mybir
from concourse.bass_utils import run_bass_kernel_spmd

DT = mybir.dt

N_CORES = 8
B = 8192
D = 512
BL = B // N_CORES          # 1024 rows per core
NT = BL // 128             # 8 i-tiles per core
HALF = B // 2              # 4096 columns per phase
TEMP = 0.07
T_INV = 1.0 / TEMP
WIN = 256                  # positive-band window width (max group size 8 << 64)

_program = None
_last_in_maps = None


FP8_SCALE = 16.0          # host multiplies features by this before fp8 cast


def _build_program(repeat=1, mov_bufs=8, e_bufs=2, es_bufs=2, ps_bufs=4):
    nc = bacc.Bacc("TRN2", target_bir_lowering=False, debug=False,
                   num_devices=N_CORES)

    vmov = nc.dram_tensor("vmov", [D, B], DT.float8e4, kind="ExternalInput").ap()
    tmov = nc.dram_tensor("tmov", [D, B], DT.float8e4, kind="ExternalInput").ap()
    ids_win = nc.dram_tensor("ids_win", [NT, WIN], DT.float32, kind="ExternalInput").ap()
    ids_loc = nc.dram_tensor("ids_loc", [128, NT], DT.float32, kind="ExternalInput").ap()
    ident = nc.dram_tensor("ident", [128, 128], DT.float32, kind="ExternalInput").ap()

    def out_t(name, w, dtype=DT.float32):
        return nc.dram_tensor(name, [BL, w], dtype, kind="ExternalOutput").ap()

    outs = {}
    for nm in ("v2t", "t2v"):
        outs[nm + "_tot"] = out_t(nm + "_tot", 2)
        outs[nm + "_pos"] = out_t(nm + "_pos", 2)
        outs[nm + "_max"] = out_t(nm + "_max", 16)
        outs[nm + "_idx"] = out_t(nm + "_idx", 16, DT.uint32)
    for nm in ("vv", "tt"):
        outs[nm + "_tot"] = out_t(nm + "_tot", 2)
        outs[nm + "_diag"] = out_t(nm + "_diag", 1)

    with tile.TileContext(nc) as tc:
        with tc.tile_pool(name="consts", bufs=1) as cpool, \
             tc.tile_pool(name="mov", bufs=mov_bufs) as mpool, \
             tc.tile_pool(name="eblk", bufs=e_bufs) as epool, \
             tc.tile_pool(name="esc", bufs=es_bufs) as escpool, \
             tc.tile_pool(name="small", bufs=3) as spool, \
             tc.tile_pool(name="gmp", bufs=2) as gmpool, \
             tc.tile_pool(name="psum", bufs=ps_bufs, space="PSUM") as pspool:

            def load_mov(mat, half):
                # pair-packed fp8 moving tiles: mkp[kp][:, 0:4096] holds
                # k-slice 2*kp, [4096:8192) holds slice 2*kp+1 (DoubleRow)
                mov_dram = tmov if mat == "t" else vmov
                mk = [mpool.tile([128, 2 * HALF], DT.float8e4, name="movk")
                      for _ in range(2)]
                # q-outer: the first compute chunk needs all k slices first
                for q in range(4):
                    for kp in range(2):
                        for sl in range(2):
                            nc.sync.dma_start(
                                mk[kp][:, sl * HALF + q * 1024:
                                       sl * HALF + (q + 1) * 1024],
                                mov_dram[(2 * kp + sl) * 128:
                                         (2 * kp + sl + 1) * 128,
                                         half * HALF + q * 1024:
                                         half * HALF + (q + 1) * 1024])
                return mk

            # phases: (moving matrix, half)
            phases = [("t", 0), ("t", 1), ("v", 0), ("v", 1)] * repeat

            # critical path first: cross stationary (vloc) + phase-0 moving
            # pair-packed: [:, 0:BL] = k-slice 2kp, [:, BL:2BL] = slice 2kp+1
            vloc = [cpool.tile([128, 2 * BL], DT.float8e4, name=f"vloc{i}")
                    for i in range(2)]
            tloc = [cpool.tile([128, 2 * BL], DT.float8e4, name=f"tloc{i}")
                    for i in range(2)]
            for kp in range(2):
                for sl in range(2):
                    nc.sync.dma_start(
                        vloc[kp][:, sl * BL:(sl + 1) * BL],
                        vmov[(2 * kp + sl) * 128:(2 * kp + sl + 1) * 128, 0:BL])
            mk0 = load_mov(*phases[0])
            for kp in range(2):
                for sl in range(2):
                    nc.sync.dma_start(
                        tloc[kp][:, sl * BL:(sl + 1) * BL],
                        tmov[(2 * kp + sl) * 128:(2 * kp + sl + 1) * 128, 0:BL])

            win = cpool.tile([128, NT * WIN], DT.float32)
            for it in range(NT):
                nc.gpsimd.dma_start(win[:, it * WIN:(it + 1) * WIN],
                                    ids_win[it:it + 1, :].partition_broadcast(128))
            idl = cpool.tile([128, NT], DT.float32)
            nc.gpsimd.dma_start(idl[:], ids_loc[:])
            idn = cpool.tile([128, 128], DT.float32)
            nc.gpsimd.dma_start(idn[:], ident[:])
            for pi, (mat, half) in enumerate(phases):
                cross = "v2t" if mat == "t" else "t2v"
                inst = "tt" if mat == "t" else "vv"
                cstat = vloc if mat == "t" else tloc
                istat = tloc if mat == "t" else vloc

                mk = mk0 if pi == 0 else load_mov(mat, half)

                def mm_group(pp, stat, it, g):
                    # fill [128, 1024] psum group g of i-tile it
                    # (fp8 DoubleRow: 2 k-slices per matmul, 256-col chunks)
                    for cc in range(4):
                        for kp in range(2):
                            nc.tensor.matmul(
                                pp[:, cc * 256:(cc + 1) * 256],
                                stat[kp].rearrange(
                                    "p (two m) -> p two m", two=2
                                )[:, :, it * 128: it * 128 + 128],
                                mk[kp].rearrange(
                                    "p (two n) -> p two n", two=2
                                )[:, :, g * 1024 + cc * 256:
                                  g * 1024 + (cc + 1) * 256],
                                start=(kp == 0), stop=(kp == 1),
                                perf_mode=mybir.MatmulPerfMode.DoubleRow)

                for it in range(NT):
                    # ---------- cross-modal i-tile (needs max/idx + pos) ----
                    e = epool.tile([128, HALF], DT.bfloat16, name="e")
                    tp = spool.tile([128, 4], DT.float32, name="tp")
                    for g in range(4):
                        pp = pspool.tile([128, 1024], DT.float32, name="pp")
                        mm_group(pp, cstat, it, g)
                        nc.scalar.activation(
                            e[:, g * 1024:(g + 1) * 1024], pp[:],
                            mybir.ActivationFunctionType.Exp,
                            bias=0.0, scale=T_INV / (FP8_SCALE * FP8_SCALE),
                            accum_out=tp[:, g:g + 1])
                    tot1 = spool.tile([128, 1], DT.float32, name="tot1")
                    nc.vector.tensor_reduce(tot1[:], tp[:],
                                            axis=mybir.AxisListType.X,
                                            op=mybir.AluOpType.add)
                    nc.gpsimd.dma_start(
                        outs[cross + "_tot"][it * 128:(it + 1) * 128, half:half + 1],
                        tot1[:])
                    # two-level argmax: 8-wide group maxes, then top-8 groups
                    gm = gmpool.tile([128, HALF // 8], DT.float32, name="gm")
                    nc.vector.tensor_reduce(
                        gm[:], e[:].rearrange("p (g k) -> p g k", k=8),
                        axis=mybir.AxisListType.X, op=mybir.AluOpType.max)
                    mx = spool.tile([128, 8], DT.float32, name="mx")
                    ix = spool.tile([128, 8], DT.uint32, name="ix")
                    nc.vector.max_with_indices(mx[:], ix[:], gm[:])
                    nc.gpsimd.dma_start(
                        outs[cross + "_max"][it * 128:(it + 1) * 128,
                                             half * 8:(half + 1) * 8], mx[:])
                    nc.gpsimd.dma_start(
                        outs[cross + "_idx"][it * 128:(it + 1) * 128,
                                             half * 8:(half + 1) * 8], ix[:])

                    # positive band: local cols [it*128-64, it*128+192) mod B
                    def mask_pos(e_lo, e_hi, w_lo, pos_col):
                        width = e_hi - e_lo
                        msk = spool.tile([128, WIN], DT.float32, name="msk")
                        junk = spool.tile([128, WIN], DT.float32, name="junk")
                        pos1 = spool.tile([128, 1], DT.float32, name="pos1")
                        nc.vector.tensor_scalar(
                            msk[:, 0:width],
                            win[:, it * WIN + w_lo: it * WIN + w_lo + width],
                            idl[:, it:it + 1], None,
                            op0=mybir.AluOpType.is_equal)
                        nc.vector.tensor_tensor(
                            junk[:, 0:width], e[:, e_lo:e_hi], msk[:, 0:width],
                            op=mybir.AluOpType.mult)
                        nc.vector.tensor_reduce(
                            pos1[:], junk[:, 0:width],
                            axis=mybir.AxisListType.X, op=mybir.AluOpType.add)
                        nc.gpsimd.dma_start(
                            outs[cross + "_pos"][it * 128:(it + 1) * 128,
                                                 pos_col:pos_col + 1], pos1[:])

                    if half == 0:
                        if it == 0:
                            mask_pos(0, 192, 64, 0)       # cols [0, 192)
                        else:
                            mask_pos(it * 128 - 64, it * 128 + 192, 0, 0)
                    elif it == 0:
                        mask_pos(HALF - 64, HALF, 0, 1)    # wrap: cols [B-64, B)

                    # ---------- instance i-tile (tot + diag only) ----------
                    tpi = spool.tile([128, 4], DT.float32, name="tpi")
                    for g in range(4):
                        pp = pspool.tile([128, 1024], DT.float32, name="pp")
                        mm_group(pp, istat, it, g)
                        es = escpool.tile([128, 1024], DT.float32, name="es")
                        nc.scalar.activation(
                            es[:], pp[:], mybir.ActivationFunctionType.Exp,
                            bias=0.0, scale=T_INV / (FP8_SCALE * FP8_SCALE),
                            accum_out=tpi[:, g:g + 1])
                        if half == 0 and g == 0:
                            junkd = spool.tile([128, 128], DT.float32, name="junkd")
                            diag1 = spool.tile([128, 1], DT.float32, name="diag1")
                            nc.vector.tensor_tensor(
                                junkd[:], es[:, it * 128:it * 128 + 128], idn[:],
                                op=mybir.AluOpType.mult)
                            nc.vector.tensor_reduce(
                                diag1[:], junkd[:],
                                axis=mybir.AxisListType.X, op=mybir.AluOpType.add)
                            nc.gpsimd.dma_start(
                                outs[inst + "_diag"][it * 128:(it + 1) * 128, 0:1],
                                diag1[:])
                    toti = spool.tile([128, 1], DT.float32, name="toti")
                    nc.vector.tensor_reduce(toti[:], tpi[:],
                                            axis=mybir.AxisListType.X,
                                            op=mybir.AluOpType.add)
                    nc.gpsimd.dma_start(
                        outs[inst + "_tot"][it * 128:(it + 1) * 128, half:half + 1],
                        toti[:])
    nc.compile()
    return nc


def _get_program():
    global _program
    if _program is None:
        _program = _build_program()
    return _program


def kernel(vision_features, text_features, match_ids):
    v = np.asarray(vision_features, dtype=np.float32)
    t = np.asarray(text_features, dtype=np.float32)
    ids = np.asarray(match_ids)

    # ---- host prep: sort by id, normalize, transpose ----
    perm = np.argsort(ids, kind="stable")
    ids_s = ids[perm].astype(np.int64)
    v_s = v[perm]
    t_s = t[perm]
    vn = (v_s / np.linalg.norm(v_s, axis=1, keepdims=True)).astype(np.float32)
    tn = (t_s / np.linalg.norm(t_s, axis=1, keepdims=True)).astype(np.float32)
    f8 = np.dtype(mybir.dt.np(DT.float8e4))
    vT = np.ascontiguousarray(vn.T * FP8_SCALE).astype(f8)   # [D, B] fp8
    tT = np.ascontiguousarray(tn.T * FP8_SCALE).astype(f8)

    # group ranges in sorted order
    ids_f = ids_s.astype(np.float32)
    change = np.nonzero(np.diff(ids_s))[0] + 1
    starts = np.concatenate([[0], change])
    ends = np.concatenate([change, [B]])
    cnt = ends - starts
    num_pos = int((cnt.astype(np.int64) ** 2).sum())
    assert cnt.max() <= 64, "positive band wider than window"

    in_maps = []
    for d in range(N_CORES):
        sl = slice(d * BL, (d + 1) * BL)
        vrot = np.roll(vT, -d * BL, axis=1)
        trot = np.roll(tT, -d * BL, axis=1)
        ids_win = np.empty((NT, WIN), np.float32)
        for it in range(NT):
            cols = (np.arange(it * 128 - 64, it * 128 + 192) + d * BL) % B
            ids_win[it] = ids_f[cols]
        ids_loc = ids_f[sl].reshape(NT, 128).T.copy()  # [128, NT]
        in_maps.append({
            "vmov": vrot, "tmov": trot,
            "ids_win": ids_win, "ids_loc": ids_loc,
            "ident": np.eye(128, dtype=np.float32),
        })

    global _last_in_maps
    _last_in_maps = in_maps
    nc = _get_program()
    res = run_bass_kernel_spmd(nc, in_maps, list(range(N_CORES)))

    def gather(name):
        return np.concatenate([res.results[c][name] for c in range(N_CORES)], axis=0)

    out = {k: gather(k) for k in
           ["v2t_tot", "v2t_pos", "v2t_max", "v2t_idx",
            "t2v_tot", "t2v_pos", "t2v_max", "t2v_idx",
            "vv_tot", "vv_diag", "tt_tot", "tt_diag"]}

    # ---- losses (all rows valid: every row has >=1 pos and >=1 neg) ----
    f64 = np.float64
    v2t_tot = out["v2t_tot"].astype(f64).sum(1)
    t2v_tot = out["t2v_tot"].astype(f64).sum(1)

    def pos_sum(a):
        # column 1 is only written for each core's first i-tile (the band
        # wrap); everything else is undefined memory — mask it out.
        s = a.astype(f64)[:, 0].copy()
        for c in range(N_CORES):
            lo = c * BL
            s[lo:lo + 128] += a[lo:lo + 128, 1].astype(f64)
        return s

    v2t_pos = pos_sum(out["v2t_pos"])
    t2v_pos = pos_sum(out["t2v_pos"])
    v2t_loss = (np.log(v2t_tot) - np.log(v2t_pos)).sum() / num_pos
    t2v_loss = (np.log(t2v_tot) - np.log(t2v_pos)).sum() / num_pos
    cross = 0.5 * (v2t_loss + t2v_loss)

    vv_tot = out["vv_tot"].astype(f64).sum(1)
    tt_tot = out["tt_tot"].astype(f64).sum(1)
    v_inst = (np.log(vv_tot) - np.log(out["vv_diag"].astype(f64)[:, 0])).mean()
    t_inst = (np.log(tt_tot) - np.log(out["tt_diag"].astype(f64)[:, 0])).mean()

    total = cross + 0.5 * v_inst + 0.5 * t_inst

    # ---- accuracy: refine argmax among the 16 device candidates ----
    core_of_row = np.repeat(np.arange(N_CORES), BL)

    def refine(idx, a_s, b_s):
        # idx: [B, 16] top-8 group indices per half (groups of 8 columns);
        # expand to the 128 member columns and take the exact-fp32 argmax.
        gidx = idx.astype(np.int64)
        gidx[:, 8:] += HALF // 8
        loc = (gidx[:, :, None] * 8 + np.arange(8)).reshape(B, 128)
        g = (loc + core_of_row[:, None] * BL) % B      # global sorted col
        best = np.empty(B, np.int64)
        for lo in range(0, B, 512):
            hi = lo + 512
            cand = b_s[g[lo:hi]]                       # [512, 128, D]
            sims = np.matmul(cand, a_s[lo:hi, :, None])[:, :, 0]
            best[lo:hi] = g[np.arange(lo, hi), sims.argmax(1)]
        return best

    v2t_pred_s = refine(out["v2t_idx"], vn, tn)
    t2v_pred_s = refine(out["t2v_idx"], tn, vn)

    # map sorted-space preds back to original indexing
    ids_orig = ids.astype(np.int64)
    order = np.argsort(ids_orig, kind="stable")
    first_occ_sorted_pos = np.searchsorted(ids_orig[order], ids_orig)
    targets = order[first_occ_sorted_pos]              # first orig idx with same id

    pred_v2t = np.empty(B, np.int64)
    pred_v2t[perm] = perm[v2t_pred_s]
    pred_t2v = np.empty(B, np.int64)
    pred_t2v[perm] = perm[t2v_pred_s]
    v2t_acc = (pred_v2t == targets).mean()
    t2v_acc = (pred_t2v == targets).mean()

    r = np.float32
    return (r(total), r(cross), r(v2t_loss), r(t2v_loss),
            r(v_inst), r(t_inst), r(v2t_acc), r(t2v_acc),
            r((v2t_acc + t2v_acc) / 2.0))



# revision 10
# speedup vs baseline: 2.5684x; 1.7778x over previous
"""DecoupledContrastiveLoss on 8 Trainium2 NeuronCores.

Strategy (data parallel over batch rows, per sharding hint):
  - Host: stable-sort rows by match_id (positive mask becomes a narrow band
    around the diagonal), L2-normalize, transpose to [D, B], quantize to
    fp8e4 (x16 scale), and ship each core a column-rotated copy (rotation
    by core*1024 puts the core's own block at local columns [0, 1024)).
  - Device (per core), all sim matmuls fp8 DoubleRow (2 k-slices/matmul):
      * v2t pass (2 halves x 8 i-tiles): sim psum -> ACT exp (1536-wide
        chunks, fused row-sum accum) -> e bf16 tiles.  DVE fold-max chains
        give per-row group maxes (f3, DMA'd to host); cross-i-tile fold-max
        gives per-column maxes (M, DMA'd to host).  Ones-matmul column sums
        of e give the t2v denominators (no t2v exp pass at all).  The
        diagonal band of e is DMA'd raw; host masks it for both pos sums.
      * vv/tt passes over only 5 of 8 column blocks (symmetric matrix):
        exp row-sum accum + ones-matmul column sums of blocks 1-3; host
        redistributes column sums as the missing row-sum pieces.
  - Host: assembles tots/pos (f64), corrects the quantized diagonal of the
    instance matrices exactly, computes losses, and refines both argmaxes
    with exact fp32 dots over the device-provided candidates.
"""
import sys

if "/opt/trn_rl_repo" not in sys.path:
    sys.path.insert(0, "/opt/trn_rl_repo")

import numpy as np

import concourse.bacc as bacc
import concourse.tile as tile
import concourse.mybir as mybir
from concourse.bass_utils import run_bass_kernel_spmd

DT = mybir.dt

N_CORES = 8
B = 8192
D = 512
BL = B // N_CORES          # 1024 rows per core
NT = BL // 128             # 8 i-tiles per core
HALF = B // 2              # 4096 columns per cross phase
IW = 5 * BL                # instance pass width: blocks 0..4 (5120 cols)
CSL, CSH = BL, 4 * BL      # instance colsum cols [1024, 4096)
TEMP = 0.07
T_INV = 1.0 / TEMP
WIN = 256                  # positive-band window width (max group size << 64)
FP8_SCALE = 16.0           # host multiplies features by this before fp8 cast
ESC = T_INV / (FP8_SCALE * FP8_SCALE)   # exp() scale on device

_program = None
_last_in_maps = None


def _build_program():
    nc = bacc.Bacc("TRN2", target_bir_lowering=False, debug=False,
                   num_devices=N_CORES)

    vmov = nc.dram_tensor("vmov", [D, B], DT.float8e4, kind="ExternalInput").ap()
    tmov = nc.dram_tensor("tmov", [D, B], DT.float8e4, kind="ExternalInput").ap()

    outs = {
        "v2t_tot": nc.dram_tensor("v2t_tot", [2 * BL, 3], DT.float32,
                                  kind="ExternalOutput").ap(),
        "vv_tot": nc.dram_tensor("vv_tot", [BL, 4], DT.float32,
                                 kind="ExternalOutput").ap(),
        "tt_tot": nc.dram_tensor("tt_tot", [BL, 4], DT.float32,
                                 kind="ExternalOutput").ap(),
        "t2v_totc": nc.dram_tensor("t2v_totc", [16, 512], DT.float32,
                                   kind="ExternalOutput").ap(),
        "vv_csc": nc.dram_tensor("vv_csc", [6, 512], DT.float32,
                                 kind="ExternalOutput").ap(),
        "tt_csc": nc.dram_tensor("tt_csc", [6, 512], DT.float32,
                                 kind="ExternalOutput").ap(),
        "mfold": nc.dram_tensor("mfold", [2 * 128, HALF], DT.bfloat16,
                                kind="ExternalOutput").ap(),
        "f1": nc.dram_tensor("f1", [2 * BL, 2048], DT.bfloat16,
                             kind="ExternalOutput").ap(),
        "band": nc.dram_tensor("band", [BL, WIN], DT.bfloat16,
                               kind="ExternalOutput").ap(),
        "wrapband": nc.dram_tensor("wrapband", [128, 64], DT.bfloat16,
                                   kind="ExternalOutput").ap(),
    }

    with tile.TileContext(nc) as tc:
        with tc.tile_pool(name="consts", bufs=1) as cpool, \
             tc.tile_pool(name="mov", bufs=4) as mpool, \
             tc.tile_pool(name="eblk", bufs=6) as epool, \
             tc.tile_pool(name="fold", bufs=3) as fpool, \
             tc.tile_pool(name="accp", bufs=2) as apool, \
             tc.tile_pool(name="mfoldp", bufs=2) as mpoolf, \
             tc.tile_pool(name="small", bufs=4) as spool, \
             tc.tile_pool(name="csrow", bufs=4) as cspool_s, \
             tc.tile_pool(name="psmain", bufs=2, space="PSUM") as pmain, \
             tc.tile_pool(name="pscs", bufs=2, space="PSUM") as pcs:

            # ---- constants / stationary ----
            ones = cpool.tile([128, 128], DT.bfloat16)
            nc.vector.memset(ones[:], 1.0)
            vloc = [cpool.tile([128, 2 * BL], DT.float8e4, name=f"vloc{i}")
                    for i in range(2)]
            tloc = [cpool.tile([128, 2 * BL], DT.float8e4, name=f"tloc{i}")
                    for i in range(2)]
            for kp in range(2):
                for sl in range(2):
                    nc.sync.dma_start(
                        vloc[kp][:, sl * BL:(sl + 1) * BL],
                        vmov[(2 * kp + sl) * 128:(2 * kp + sl + 1) * 128, 0:BL])
            for kp in range(2):
                for sl in range(2):
                    nc.sync.dma_start(
                        tloc[kp][:, sl * BL:(sl + 1) * BL],
                        tmov[(2 * kp + sl) * 128:(2 * kp + sl + 1) * 128, 0:BL])

            def load_mov(mat, c0, width):
                # pair-packed fp8 moving tiles for DoubleRow:
                # mk[kp][:, 0:width] = k-slice 2kp, [width:2w) = slice 2kp+1
                mov_dram = tmov if mat == "t" else vmov
                mk = [mpool.tile([128, 2 * width], DT.float8e4, name="movk")
                      for _ in range(2)]
                nq = width // 1024
                for q in range(nq):
                    for kp in range(2):
                        for sl in range(2):
                            nc.sync.dma_start(
                                mk[kp][:, sl * width + q * 1024:
                                       sl * width + (q + 1) * 1024],
                                mov_dram[(2 * kp + sl) * 128:
                                         (2 * kp + sl + 1) * 128,
                                         c0 + q * 1024: c0 + (q + 1) * 1024])
                return mk

            def mm_chunk(pp, plo, stat, mk, width, it, c_lo, c_hi):
                # fill psum pp[:, plo:plo+(c_hi-c_lo)] with sim cols
                # [c_lo, c_hi) of i-tile `it` (fp8 DoubleRow, 256-col chunks)
                for ci, c in enumerate(range(c_lo, c_hi, 256)):
                    for kp in range(2):
                        nc.tensor.matmul(
                            pp[:, plo + ci * 256: plo + (ci + 1) * 256],
                            stat[kp].rearrange(
                                "p (two m) -> p two m", two=2
                            )[:, :, it * 128: it * 128 + 128],
                            mk[kp].rearrange(
                                "p (two n) -> p two n", two=2
                            )[:, :, c: c + 256],
                            start=(kp == 0), stop=(kp == 1),
                            perf_mode=mybir.MatmulPerfMode.DoubleRow)

            def colsum_burst(acc, width, out_dram, out_row0):
                # ones-matmul column sums of the bf16 running-sum tile,
                # 512 cols per chunk -> DVE copy -> DRAM
                for ci in range(width // 512):
                    c = ci * 512
                    cs = pcs.tile([128, 512], DT.float32, name="cs")
                    nc.tensor.matmul(cs[:], ones[:], acc[:, c:c + 512],
                                     start=True, stop=True)
                    csr = cspool_s.tile([1, 512], DT.float32, name="csr")
                    nc.vector.tensor_scalar(csr[:], cs[0:1, :], 1.0, None,
                                            op0=mybir.AluOpType.mult)
                    nc.gpsimd.dma_start(
                        out_dram[out_row0 + ci: out_row0 + ci + 1, :], csr[:])

            # widths of the ACT chunks tiling one i-tile's columns
            def chunks_of(width):
                out, c = [], 0
                while c < width:
                    w = min(1536, width - c)
                    out.append((c, w))
                    c += w
                return out

            pending_cs = None     # deferred colsum closure from previous phase

            def run_pass(name, mat, stat, c0, width, acc_lo, acc_hi,
                         tot_dram, tot_row0, do_maxes):
                nonlocal pending_cs
                mk = load_mov(mat, c0, width)
                acc = apool.tile([128, acc_hi - acc_lo], DT.bfloat16,
                                 name="acc")
                mfold = None
                e_prev = None
                for it in range(NT):
                    e = epool.tile([128, IW], DT.bfloat16, name="e")
                    tp = spool.tile([128, 4], DT.float32, name="tp")
                    nchunks = chunks_of(width)
                    for gi, (glo, gw) in enumerate(nchunks):
                        pp = pmain.tile([128, 1536], DT.float32, name="pp")
                        mm_chunk(pp, 0, stat, mk, width, it, glo, glo + gw)
                        nc.scalar.activation(
                            e[:, glo:glo + gw], pp[:, 0:gw],
                            mybir.ActivationFunctionType.Exp,
                            bias=0.0, scale=ESC, accum_out=tp[:, gi:gi + 1])
                    nc.gpsimd.dma_start(
                        tot_dram[tot_row0 + it * 128:
                                 tot_row0 + (it + 1) * 128, 0:len(nchunks)],
                        tp[:, 0:len(nchunks)])

                    # running column sums (bf16): acc = e0+e1, then += e
                    if it == 1:
                        nc.vector.tensor_tensor(
                            acc[:], e_prev[:, acc_lo:acc_hi],
                            e[:, acc_lo:acc_hi], op=mybir.AluOpType.add)
                    elif it > 1:
                        nc.vector.tensor_tensor(
                            acc[:], acc[:], e[:, acc_lo:acc_hi],
                            op=mybir.AluOpType.add)
                    if do_maxes:
                        half = c0 // HALF
                        # per-row fold 4096 -> 2048 (bf16, 2x DVE) -> host
                        f1 = fpool.tile([128, 2048], DT.bfloat16, name="f1")
                        nc.vector.tensor_tensor(
                            f1[:], e[:, 0:2048], e[:, 2048:4096],
                            op=mybir.AluOpType.max)
                        nc.gpsimd.dma_start(
                            outs["f1"][half * BL + it * 128:
                                       half * BL + (it + 1) * 128, :], f1[:])
                        # per-column fold across i-tiles
                        if it == 0:
                            mfold = mpoolf.tile([128, HALF], DT.bfloat16,
                                                name="mfold")
                        if it == 1:
                            nc.vector.tensor_tensor(
                                mfold[:], e_prev[:, 0:HALF], e[:, 0:HALF],
                                op=mybir.AluOpType.max)
                        elif it > 1:
                            nc.vector.tensor_tensor(
                                mfold[:], mfold[:], e[:, 0:HALF],
                                op=mybir.AluOpType.max)
                        # diagonal band -> host (v2t pass only)
                        if half == 0:
                            if it == 0:
                                nc.gpsimd.dma_start(outs["band"][0:128, 64:256],
                                                    e[:, 0:192])
                            else:
                                nc.gpsimd.dma_start(
                                    outs["band"][it * 128:(it + 1) * 128, :],
                                    e[:, it * 128 - 64: it * 128 + 192])
                        elif it == 0:
                            nc.gpsimd.dma_start(outs["wrapband"][:],
                                                e[:, HALF - 64:HALF])
                    e_prev = e
                if do_maxes:
                    half = c0 // HALF
                    nc.gpsimd.dma_start(
                        outs["mfold"][half * 128:(half + 1) * 128, :],
                        mfold[:])

                # run the deferred colsum of the previous phase now (it
                # overlaps this phase's tail instead of stalling PE/ACT)
                if pending_cs is not None:
                    pending_cs()
                    pending_cs = None
                return acc

            # ---- phase sequence ----
            # v2t halves (cross), then tt, vv (instance, 5 blocks wide)
            acc0 = run_pass("v2t0", "t", vloc, 0, HALF, 0, HALF,
                            outs["v2t_tot"], 0, True)
            pending_cs = (lambda a=acc0: colsum_burst(
                a, HALF, outs["t2v_totc"], 0))

            acc1 = run_pass("v2t1", "t", vloc, HALF, HALF, 0, HALF,
                            outs["v2t_tot"], BL, True)
            pending_cs = (lambda a=acc1: colsum_burst(
                a, HALF, outs["t2v_totc"], 8))

            acc_tt = run_pass("tt", "t", tloc, 0, IW, CSL, CSH,
                              outs["tt_tot"], 0, False)
            pending_cs = (lambda a=acc_tt: colsum_burst(
                a, CSH - CSL, outs["tt_csc"], 0))

            acc_vv = run_pass("vv", "v", vloc, 0, IW, CSL, CSH,
                              outs["vv_tot"], 0, False)
            if pending_cs is not None:
                pending_cs()
                pending_cs = None
            colsum_burst(acc_vv, CSH - CSL, outs["vv_csc"], 0)
    nc.compile()
    return nc


def _get_program():
    global _program
    if _program is None:
        _program = _build_program()
    return _program


def kernel(vision_features, text_features, match_ids):
    v = np.asarray(vision_features, dtype=np.float32)
    t = np.asarray(text_features, dtype=np.float32)
    ids = np.asarray(match_ids)

    # ---- host prep: sort by id, normalize, quantize, rotate ----
    perm = np.argsort(ids, kind="stable")
    ids_s = ids[perm].astype(np.int64)
    v_s = v[perm]
    t_s = t[perm]
    vn = (v_s / np.linalg.norm(v_s, axis=1, keepdims=True)).astype(np.float32)
    tn = (t_s / np.linalg.norm(t_s, axis=1, keepdims=True)).astype(np.float32)
    f8 = np.dtype(mybir.dt.np(DT.float8e4))
    vT = np.ascontiguousarray(vn.T * FP8_SCALE).astype(f8)   # [D, B] fp8
    tT = np.ascontiguousarray(tn.T * FP8_SCALE).astype(f8)

    ids_f = ids_s.astype(np.float32)
    change = np.nonzero(np.diff(ids_s))[0] + 1
    starts = np.concatenate([[0], change])
    ends = np.concatenate([change, [B]])
    cnt = ends - starts
    num_pos = int((cnt.astype(np.int64) ** 2).sum())
    assert cnt.max() <= 64, "positive band wider than window"

    in_maps = []
    for d in range(N_CORES):
        vrot = np.roll(vT, -d * BL, axis=1)
        trot = np.roll(tT, -d * BL, axis=1)
        in_maps.append({"vmov": vrot, "tmov": trot})

    global _last_in_maps
    _last_in_maps = in_maps
    nc = _get_program()
    res = run_bass_kernel_spmd(nc, in_maps, list(range(N_CORES)))
    R = [res.results[c] for c in range(N_CORES)]
    f64 = np.float64

    # quantized (device-visible) normalized features, de-scaled
    vq = vT.astype(np.float32).T / FP8_SCALE          # [B, D] (sorted order)
    tq = tT.astype(np.float32).T / FP8_SCALE

    # ---- v2t / t2v tot ----
    v2t_tot = np.concatenate(
        [R[c]["v2t_tot"].astype(f64).reshape(2, BL, 3).sum(2).sum(0)[None]
         for c in range(N_CORES)]).reshape(B)
    # t2v tot: per-core column sums in rotated space -> derotate, add
    t2v_tot = np.zeros(B, f64)
    for c in range(N_CORES):
        cs = R[c]["t2v_totc"].astype(f64).reshape(B)
        t2v_tot += np.roll(cs, c * BL)

    # ---- band -> pos sums (host-masked) ----
    # band rows: core c, i-tile it covers its 128 rows; columns are local
    # [it*128-64, it*128+192) (it=0: only [0,192) at band[:,64:], plus the
    # wrap piece [B-64, B) in wrapband)
    v2t_pos = np.zeros(B, f64)
    t2v_pos = np.zeros(B, f64)
    for c in range(N_CORES):
        band = R[c]["band"].astype(f64)
        wrap = R[c]["wrapband"].astype(f64)
        for it in range(NT):
            rows = c * BL + it * 128 + np.arange(128)
            lo = it * 128 - 64
            cols = (np.arange(lo, lo + WIN) + c * BL) % B
            seg = band[it * 128:(it + 1) * 128, :].copy()
            if it == 0:
                # local cols [-64, 0) came from wrapband
                seg[:, 0:64] = wrap
            m = ids_f[rows][:, None] == ids_f[cols][None, :]
            contrib = np.where(m, seg, 0.0)
            v2t_pos[rows] += contrib.sum(1)
            np.add.at(t2v_pos, cols, contrib.sum(0))

    v2t_loss = (np.log(v2t_tot) - np.log(v2t_pos)).sum() / num_pos
    t2v_loss = (np.log(t2v_tot) - np.log(t2v_pos)).sum() / num_pos
    cross = 0.5 * (v2t_loss + t2v_loss)

    # ---- instance tots: own rows + redistributed column sums ----
    def inst_tot(key_tot, key_csc, feats_q):
        tot = np.concatenate(
            [R[c][key_tot].astype(f64).sum(1) for c in range(N_CORES)])
        extra = np.zeros(B, f64)
        for c in range(N_CORES):
            cs = R[c][key_csc].astype(f64).reshape(3 * BL)   # local [1024,4096)
            g = (np.arange(CSL, CSH) + c * BL) % B
            extra[g] += cs
        tot = tot + extra
        # exact diagonal correction: replace quantized diag with exp(1/T)
        dq = (feats_q ** 2).sum(1) * T_INV
        tot = tot - np.exp(dq) + np.exp(T_INV)
        return tot

    vv_tot = inst_tot("vv_tot", "vv_csc", vq)
    tt_tot = inst_tot("tt_tot", "tt_csc", tq)
    v_inst = (np.log(vv_tot) - T_INV).mean()
    t_inst = (np.log(tt_tot) - T_INV).mean()

    total = cross + 0.5 * v_inst + 0.5 * t_inst

    # ---- v2t argmax: top-K fold pairs per half -> exact dots ----
    f1 = np.concatenate([R[c]["f1"].reshape(2, BL, 2048) for c in range(N_CORES)],
                        axis=1).astype(np.float32)           # [2, B, 2048]
    K = 8
    cand = np.empty((B, 2 * K * 2), np.int64)
    for h in range(2):
        topg = np.argpartition(-f1[h], K, axis=1)[:, :K]     # [B, K] pairs
        # pair s -> cols {s, s + 2048}, + h*4096 ; rotate per core
        loc = (topg[:, :, None] + np.array([0, 2048])[None, None, :]
               ).reshape(B, K * 2) + h * HALF
        cand[:, h * K * 2:(h + 1) * K * 2] = loc
    core_of_row = np.repeat(np.arange(N_CORES), BL)
    gcand = (cand + core_of_row[:, None] * BL) % B
    v2t_pred_s = np.empty(B, np.int64)
    for lo in range(0, B, 512):
        hi = lo + 512
        sims = np.einsum("rkd,rd->rk", tn[gcand[lo:hi]], vn[lo:hi])
        v2t_pred_s[lo:hi] = gcand[np.arange(lo, hi), sims.argmax(1)]

    # ---- t2v argmax: per-column fold -> best (core, partition), 8 i-tiles ----
    mf = np.stack([R[c]["mfold"].reshape(2, 128, HALF).transpose(1, 0, 2)
                   .reshape(128, B) for c in range(N_CORES)])  # [8, 128, B] rot
    # derotate columns: core c local col x -> global (x + c*BL) % B
    for c in range(N_CORES):
        mf[c] = np.roll(mf[c], c * BL, axis=1)
    mc = mf.astype(np.float32)
    best_p = mc.reshape(N_CORES * 128, B).argmax(0)           # core*128+p
    bc, bp = best_p // 128, best_p % 128
    # candidate rows: (core bc, i-tile 0..7, partition bp)
    crows = (bc[:, None] * BL + np.arange(NT)[None, :] * 128 + bp[:, None])
    t2v_pred_s = np.empty(B, np.int64)
    for lo in range(0, B, 1024):
        hi = lo + 1024
        sims = np.einsum("rkd,rd->rk", vn[crows[lo:hi]], tn[lo:hi])
        t2v_pred_s[lo:hi] = crows[np.arange(lo, hi), sims.argmax(1)]

    # ---- map sorted-space preds back to original indexing ----
    ids_orig = ids.astype(np.int64)
    order = np.argsort(ids_orig, kind="stable")
    first_occ = np.searchsorted(ids_orig[order], ids_orig)
    targets = order[first_occ]

    pred_v2t = np.empty(B, np.int64)
    pred_v2t[perm] = perm[v2t_pred_s]
    pred_t2v = np.empty(B, np.int64)
    pred_t2v[perm] = perm[t2v_pred_s]
    v2t_acc = (pred_v2t == targets).mean()
    t2v_acc = (pred_t2v == targets).mean()

    r = np.float32
    return (r(total), r(cross), r(v2t_loss), r(t2v_loss),
            r(v_inst), r(t_inst), r(v2t_acc), r(t2v_acc),
            r((v2t_acc + t2v_acc) / 2.0))
